# revision 1
# baseline (speedup 1.0000x reference)
"""Trainium2 Bass kernel for nn_CNNBranch (ragged multi-branch patch projection).

Semantics (matching the reference):
  per sample b: w = WINDOW_SIZES[branch_idx[b]], S = T // w
    tokens[b, s, :] = signal[b, s*w:(s+1)*w] @ W[branch].T + bias[branch]   for s < n_valid[b]
    tokens[b, s, :] = 0                                                     otherwise
    padding_mask[b, s] = s < n_valid[b]
  n_valid[b] = min(sum(signal_mask[b]) // w, S, S_max)

Strategy: the host packs ONLY the valid token rows (across all samples of a
branch — rows from different samples share the same weights, so they can share
128-row matmul tiles) into the transposed lhsT layout the PE needs. All 8 cores
run one identical SPMD program on equal tile counts; the device does fp32
matmul (K=128 chunks accumulated in PSUM) + bias add, and streams compact
output tiles back. The host scatters valid rows into the zero output.
"""

import math
import numpy as np

import concourse.bass as bass
import concourse.bacc as bacc
import concourse.tile as tile
import concourse.mybir as mybir
from concourse.bass_utils import run_bass_kernel_spmd

WINDOW_SIZES = (128, 256, 512)
E = 512
B, T = 128, 65536
S_MAX = 512
P = 128
N_CORES = 8
R_BY_BRANCH = (1, 2, 4)          # w // 128
WCHUNK_BASE = (0, 1, 3)          # start index of each branch's K-chunks in the packed weight tensor
N_WCHUNKS = 7

_program_cache: dict = {}


def _build_program(Q: tuple):
    """One SPMD program for per-core tile counts Q=(Q0,Q1,Q2)."""
    Q0, Q1, Q2 = Q
    n_items = Q0 + Q1 + Q2
    c_total = Q0 * 1 + Q1 * 2 + Q2 * 4

    nc = bacc.Bacc("TRN2", target_bir_lowering=False, debug=False)
    f32 = mybir.dt.float32

    inp = nc.dram_tensor("inp", [P, c_total * P], f32, kind="ExternalInput").ap()
    wch = nc.dram_tensor("wch", [P, N_WCHUNKS * E], f32, kind="ExternalInput").ap()
    bias = nc.dram_tensor("bias", [P, 3 * E], f32, kind="ExternalInput").ap()
    out = nc.dram_tensor("out", [n_items * P, E], f32, kind="ExternalOutput").ap()

    # item schedule: (branch, col_off in 128-col units, out_row)
    items = []
    col = 0
    for b in range(3):
        r = R_BY_BRANCH[b]
        for _ in range(Q[b]):
            items.append((b, col))
            col += r

    with tile.TileContext(nc) as tc:
        with (
            tc.tile_pool(name="const", bufs=1) as cpool,
            tc.tile_pool(name="inbig", bufs=1) as ipool,
            tc.tile_pool(name="psum", bufs=4, space="PSUM") as ppool,
            tc.tile_pool(name="outs", bufs=4) as opool,
        ):
            w_tile = cpool.tile([P, N_WCHUNKS * E], f32, tag="w")
            b_tile = cpool.tile([P, 3 * E], f32, tag="b")
            nc.sync.dma_start(w_tile[:], wch[:])
            nc.sync.dma_start(b_tile[:], bias[:])

            in_tile = ipool.tile([P, c_total * P], f32, tag="in")
            nc.sync.dma_start(in_tile[:], inp[:])

            for i, (b, col) in enumerate(items):
                r = R_BY_BRANCH[b]
                wb = WCHUNK_BASE[b]
                ps = ppool.tile([P, E], f32, tag="ps")
                for cc in range(r):
                    nc.tensor.matmul(
                        ps[:],
                        in_tile[:, (col + cc) * P:(col + cc + 1) * P],
                        w_tile[:, (wb + cc) * E:(wb + cc + 1) * E],
                        start=(cc == 0),
                        stop=(cc == r - 1),
                    )
                ot = opool.tile([P, E], f32, tag="ot")
                nc.vector.tensor_add(ot[:], ps[:], b_tile[:, b * E:(b + 1) * E])
                nc.scalar.dma_start(out[i * P:(i + 1) * P, :], ot[:])

    nc.compile()
    return nc


def _get_program(Q: tuple):
    key = tuple(Q)
    if key not in _program_cache:
        _program_cache[key] = _build_program(key)
    return _program_cache[key]


def _pack_weights(Ws, bs):
    wch = np.empty((P, N_WCHUNKS * E), dtype=np.float32)
    i = 0
    for b in range(3):
        Wb = np.asarray(Ws[b], dtype=np.float32)
        for cc in range(R_BY_BRANCH[b]):
            wch[:, i * E:(i + 1) * E] = Wb[:, cc * P:(cc + 1) * P].T
            i += 1
    bias = np.empty((P, 3 * E), dtype=np.float32)
    for b in range(3):
        bias[:, b * E:(b + 1) * E] = np.asarray(bs[b], dtype=np.float32)[None, :]
    return wch, bias


def kernel(signal, signal_mask, branch_idx, W0, b0, W1, b1, W2, b2):
    signal = np.asarray(signal, dtype=np.float32)
    signal_mask = np.asarray(signal_mask)
    branch_idx = np.asarray(branch_idx, dtype=np.int32)

    lengths = signal_mask.sum(axis=1, dtype=np.int64)
    w_of = np.asarray(WINDOW_SIZES, dtype=np.int64)[branch_idx]
    S_of = T // w_of
    n_valid = np.minimum(lengths // w_of, np.minimum(S_of, S_MAX))

    tokens = np.zeros((B, S_MAX, E), dtype=np.float32)
    padding_mask = np.arange(S_MAX, dtype=np.int64)[None, :] < n_valid[:, None]

    # ---- host packing: per branch, gather valid rows, split across cores ----
    per_core_in = []      # per core: list of (128,128) chunk blocks in item order
    Q = []
    scatter = [[] for _ in range(N_CORES)]   # per core: (out_row_start, flat_token_idx array)
    row_base = 0
    branch_blocks = [[] for _ in range(N_CORES)]  # chunks per core per branch, in order
    for b in range(3):
        w = WINDOW_SIZES[b]
        r = R_BY_BRANCH[b]
        samples = np.where(branch_idx == b)[0]
        nv = n_valid[samples]
        V = int(nv.sum())
        if V == 0:
            Q.append(0)
            continue
        sm = np.repeat(samples, nv)
        starts = np.cumsum(nv) - nv
        ss = np.arange(V, dtype=np.int64) - np.repeat(starts, nv)
        rpc = math.ceil(V / N_CORES)          # rows per core (last core may have fewer)
        Qb = math.ceil(rpc / P)
        Q.append(Qb)
        rows_pad = N_CORES * Qb * P
        # gather all valid rows: A[j] = signal[sm[j], ss[j]*w : ss[j]*w + w]
        A = signal[sm[:, None], (ss * w)[:, None] + np.arange(w)[None, :]]
        A = np.concatenate([A, np.zeros((rows_pad - V, w), np.float32)], axis=0)
        # NOTE rows are dealt to cores in contiguous blocks of Qb*P
        A = A.reshape(N_CORES, Qb, P, r, P)
        # chunk[core, tile, cc][k, j] = A[core, tile, j, cc, k]
        chunks = np.ascontiguousarray(A.transpose(0, 1, 3, 4, 2))  # (cores, Qb, r, Pk, Pj)
        gflat = sm * S_MAX + ss
        for c in range(N_CORES):
            branch_blocks[c].append(chunks[c].reshape(Qb * r, P, P))
            lo = min(c * Qb * P, V)
            hi = min((c + 1) * Qb * P, V)
            if hi > lo:
                scatter[c].append((row_base, gflat[lo:hi]))
        row_base += Qb * P

    Q = tuple(Q + [0] * (3 - len(Q)))
    n_items = sum(Q)
    if n_items == 0:
        return tokens, padding_mask

    c_total = Q[0] + 2 * Q[1] + 4 * Q[2]
    for c in range(N_CORES):
        blk = np.concatenate(branch_blocks[c], axis=0)  # (c_total, P, P)
        per_core_in.append(
            np.ascontiguousarray(blk.transpose(1, 0, 2)).reshape(P, c_total * P)
        )

    wch, bias = _pack_weights((W0, W1, W2), (b0, b1, b2))

    nc = _get_program(Q)
    in_maps = [
        {"inp": per_core_in[c], "wch": wch, "bias": bias} for c in range(N_CORES)
    ]
    res = run_bass_kernel_spmd(nc, in_maps, list(range(N_CORES)))

    tok_flat = tokens.reshape(B * S_MAX, E)
    for c in range(N_CORES):
        out_c = res.results[c]["out"]
        for row_start, gidx in scatter[c]:
            tok_flat[gidx] = out_c[row_start:row_start + len(gidx)]
    return tokens, padding_mask


# revision 3
# speedup vs baseline: 1.5920x; 1.5920x over previous
"""Trainium2 Bass kernel for nn_CNNBranch (ragged multi-branch patch projection).

Semantics (matching the reference):
  per sample b: w = WINDOW_SIZES[branch_idx[b]], S = T // w
    tokens[b, s, :] = signal[b, s*w:(s+1)*w] @ W[branch].T + bias[branch]   for s < n_valid[b]
    tokens[b, s, :] = 0                                                     otherwise
    padding_mask[b, s] = s < n_valid[b]
  n_valid[b] = min(sum(signal_mask[b]) // w, S, S_max)

Strategy: the host packs ONLY the valid token rows (across all samples of a
branch — rows from different samples share the same weights, so they can share
128-row matmul tiles) into the transposed lhsT layout the PE needs. All 8 cores
run one identical SPMD program on equal tile counts; the device does fp32
matmul (K=128 chunks accumulated in PSUM) + bias add, and streams grouped
output tiles back. The host scatters valid rows into the zero output.
"""

import math
import os
import numpy as np

import concourse.bass as bass
import concourse.bacc as bacc
import concourse.tile as tile
import concourse.mybir as mybir
from concourse.bass_utils import run_bass_kernel_spmd

WINDOW_SIZES = (128, 256, 512)
E = 512
B, T = 128, 65536
S_MAX = 512
P = 128
N_CORES = 8
R_BY_BRANCH = (1, 2, 4)          # w // 128
WCHUNK_BASE = (0, 1, 3)          # start of each branch's K-chunks in the packed weight tensor
N_WCHUNKS = 7
LOAD_GROUP_CHUNKS = 8            # ~512KB per input load DMA
STORE_GROUP_ITEMS = 4            # ~1MB per output store DMA

MM_DT_NAME = os.environ.get("BASS_MM_DT", "float32")

_program_cache: dict = {}


def _item_schedule(Q):
    """[(branch, chunk_col, item_idx)] in execution order; uniform across cores."""
    items = []
    col = 0
    for b in range(3):
        for _ in range(Q[b]):
            items.append((b, col, len(items)))
            col += R_BY_BRANCH[b]
    return items, col


def _build_program(Q: tuple):
    """One SPMD program for per-core tile counts Q=(Q0,Q1,Q2)."""
    items, c_total = _item_schedule(Q)
    n_items = len(items)

    nc = bacc.Bacc("TRN2", target_bir_lowering=False, debug=False)
    f32 = mybir.dt.float32
    mm_dt = getattr(mybir.dt, MM_DT_NAME)

    inp = nc.dram_tensor("inp", [P, c_total * P], mm_dt, kind="ExternalInput").ap()
    wch = nc.dram_tensor("wch", [P, N_WCHUNKS * E], mm_dt, kind="ExternalInput").ap()
    bias = nc.dram_tensor("bias", [P, 3 * E], f32, kind="ExternalInput").ap()
    out = nc.dram_tensor("out", [P, n_items * E], f32, kind="ExternalOutput").ap()

    # group items into input-load groups of <= LOAD_GROUP_CHUNKS chunks
    lgroups = []  # (item_lo, item_hi, col_lo, col_hi)
    lo = 0
    while lo < n_items:
        hi = lo
        while hi < n_items:
            r = R_BY_BRANCH[items[hi][0]]
            used = (items[hi][1] + r) - items[lo][1]
            if used > LOAD_GROUP_CHUNKS and hi > lo:
                break
            hi += 1
        col_lo = items[lo][1]
        col_hi = items[hi][1] if hi < n_items else c_total
        lgroups.append((lo, hi, col_lo, col_hi))
        lo = hi

    G = STORE_GROUP_ITEMS
    n_sgroups = math.ceil(n_items / G)

    with tile.TileContext(nc) as tc:
        with (
            tc.tile_pool(name="const", bufs=1) as cpool,
            tc.tile_pool(name="inp_g", bufs=len(lgroups)) as ipool,
            tc.tile_pool(name="psum", bufs=4, space="PSUM") as ppool,
            tc.tile_pool(name="outs", bufs=3) as opool,
        ):
            w_tile = cpool.tile([P, N_WCHUNKS * E], mm_dt, tag="w")
            b_tile = cpool.tile([P, 3 * E], f32, tag="b")
            nc.sync.dma_start(w_tile[:], wch[:])
            nc.sync.dma_start(b_tile[:], bias[:])

            in_tiles = []
            for (ilo, ihi, clo, chi) in lgroups:
                it = ipool.tile([P, (chi - clo) * P], mm_dt, tag="in")
                nc.sync.dma_start(it[:], inp[:, clo * P:chi * P])
                in_tiles.append(it)

            for sg in range(n_sgroups):
                g_lo = sg * G
                g_hi = min(g_lo + G, n_items)
                st = opool.tile([P, (g_hi - g_lo) * E], f32, tag="st")
                for i in range(g_lo, g_hi):
                    b, col, _ = items[i]
                    r = R_BY_BRANCH[b]
                    wb = WCHUNK_BASE[b]
                    gi = next(k for k, (lo2, hi2, _, _) in enumerate(lgroups)
                              if lo2 <= i < hi2)
                    it = in_tiles[gi]
                    cbase = col - lgroups[gi][2]
                    ps = ppool.tile([P, E], f32, tag="ps")
                    for cc in range(r):
                        nc.tensor.matmul(
                            ps[:],
                            it[:, (cbase + cc) * P:(cbase + cc + 1) * P],
                            w_tile[:, (wb + cc) * E:(wb + cc + 1) * E],
                            start=(cc == 0),
                            stop=(cc == r - 1),
                        )
                    nc.vector.tensor_add(
                        st[:, (i - g_lo) * E:(i - g_lo + 1) * E],
                        ps[:],
                        b_tile[:, b * E:(b + 1) * E],
                    )
                nc.scalar.dma_start(out[:, g_lo * E:g_hi * E], st[:])

    nc.compile()
    return nc


def _get_program(Q: tuple):
    key = (tuple(Q), MM_DT_NAME)
    if key not in _program_cache:
        _program_cache[key] = _build_program(tuple(Q))
    return _program_cache[key]


def _pack_weights(Ws, bs):
    wch = np.empty((P, N_WCHUNKS * E), dtype=np.float32)
    i = 0
    for b in range(3):
        Wb = np.asarray(Ws[b], dtype=np.float32)
        for cc in range(R_BY_BRANCH[b]):
            wch[:, i * E:(i + 1) * E] = Wb[:, cc * P:(cc + 1) * P].T
            i += 1
    bias = np.empty((P, 3 * E), dtype=np.float32)
    for b in range(3):
        bias[:, b * E:(b + 1) * E] = np.asarray(bs[b], dtype=np.float32)[None, :]
    return wch, bias


def kernel(signal, signal_mask, branch_idx, W0, b0, W1, b1, W2, b2):
    signal = np.asarray(signal, dtype=np.float32)
    signal_mask = np.asarray(signal_mask)
    branch_idx = np.asarray(branch_idx, dtype=np.int32)

    lengths = signal_mask.sum(axis=1, dtype=np.int64)
    w_of = np.asarray(WINDOW_SIZES, dtype=np.int64)[branch_idx]
    S_of = T // w_of
    n_valid = np.minimum(lengths // w_of, np.minimum(S_of, S_MAX))

    tokens = np.zeros((B, S_MAX, E), dtype=np.float32)
    padding_mask = np.arange(S_MAX, dtype=np.int64)[None, :] < n_valid[:, None]

    # ---- host packing: per branch, gather valid rows, split across cores ----
    Q = []
    scatter = [[] for _ in range(N_CORES)]   # per core: (item_start, flat_token_idx array)
    item_base = 0
    branch_blocks = [[] for _ in range(N_CORES)]  # chunks per core per branch, in order
    for b in range(3):
        w = WINDOW_SIZES[b]
        r = R_BY_BRANCH[b]
        samples = np.where(branch_idx == b)[0]
        nv = n_valid[samples]
        V = int(nv.sum())
        if V == 0:
            Q.append(0)
            continue
        sm = np.repeat(samples, nv)
        starts = np.cumsum(nv) - nv
        ss = np.arange(V, dtype=np.int64) - np.repeat(starts, nv)
        rpc = math.ceil(V / N_CORES)          # rows per core (last core may have fewer)
        Qb = math.ceil(rpc / P)
        Q.append(Qb)
        rows_pad = N_CORES * Qb * P
        # gather all valid rows: A[j] = signal[sm[j], ss[j]*w : ss[j]*w + w]
        A = signal[sm[:, None], (ss * w)[:, None] + np.arange(w)[None, :]]
        A = np.concatenate([A, np.zeros((rows_pad - V, w), np.float32)], axis=0)
        # rows are dealt to cores in contiguous blocks of Qb*P
        A = A.reshape(N_CORES, Qb, P, r, P)
        # chunk[core, tile, cc][k, j] = A[core, tile, j, cc, k]
        chunks = np.ascontiguousarray(A.transpose(0, 1, 3, 4, 2))  # (cores, Qb, r, Pk, Pj)
        gflat = sm * S_MAX + ss
        for c in range(N_CORES):
            branch_blocks[c].append(chunks[c].reshape(Qb * r, P, P))
            lo = min(c * Qb * P, V)
            hi = min((c + 1) * Qb * P, V)
            if hi > lo:
                scatter[c].append((item_base, gflat[lo:hi]))
        item_base += Qb
    Q = tuple(Q)
    n_items = sum(Q)
    if n_items == 0:
        return tokens, padding_mask

    c_total = Q[0] + 2 * Q[1] + 4 * Q[2]
    per_core_in = []
    for c in range(N_CORES):
        blk = np.concatenate(branch_blocks[c], axis=0)  # (c_total, P, P)
        per_core_in.append(
            np.ascontiguousarray(blk.transpose(1, 0, 2)).reshape(P, c_total * P)
        )

    wch, bias = _pack_weights((W0, W1, W2), (b0, b1, b2))

    nc = _get_program(Q)
    in_maps = [
        {"inp": per_core_in[c], "wch": wch, "bias": bias} for c in range(N_CORES)
    ]
    res = run_bass_kernel_spmd(nc, in_maps, list(range(N_CORES)))

    tok_flat = tokens.reshape(B * S_MAX, E)
    for c in range(N_CORES):
        out_c = res.results[c]["out"]   # (P, n_items*E): item i rows at [:, i*E:(i+1)*E]
        for item_start, gidx in scatter[c]:
            n = len(gidx)
            nt = (n + P - 1) // P
            blk = out_c[:, item_start * E:(item_start + nt) * E]
            rows = np.ascontiguousarray(
                blk.reshape(P, nt, E).transpose(1, 0, 2)
            ).reshape(nt * P, E)
            tok_flat[gidx] = rows[:n]
    return tokens, padding_mask


# revision 6
# speedup vs baseline: 1.6806x; 1.0557x over previous
"""Trainium2 Bass kernel for nn_CNNBranch (ragged multi-branch patch projection).

Semantics (matching the reference):
  per sample b: w = WINDOW_SIZES[branch_idx[b]], S = T // w
    tokens[b, s, :] = signal[b, s*w:(s+1)*w] @ W[branch].T + bias[branch]   for s < n_valid[b]
    tokens[b, s, :] = 0                                                     otherwise
    padding_mask[b, s] = s < n_valid[b]
  n_valid[b] = min(sum(signal_mask[b]) // w, S, S_max)

Strategy: the host packs ONLY the valid token rows (across all samples of a
branch — rows from different samples share the same weights, so they can share
128-row matmul tiles) into the transposed lhsT layout the PE needs. All 8 cores
run one identical SPMD program on equal tile counts; the device does fp32
matmul (K=128 chunks accumulated in PSUM) + bias add, and streams grouped
output tiles back. The host scatters valid rows into the zero output.
"""

import math
import os
import numpy as np

import concourse.bass as bass
import concourse.bacc as bacc
import concourse.tile as tile
import concourse.mybir as mybir
from concourse.bass_utils import run_bass_kernel_spmd

WINDOW_SIZES = (128, 256, 512)
E = 512
B, T = 128, 65536
S_MAX = 512
P = 128
N_CORES = 8
R_BY_BRANCH = (1, 2, 4)          # w // 128
WCHUNK_BASE = (0, 1, 3)          # start of each branch's K-chunks in the packed weight tensor
N_WCHUNKS = 7
LOAD_GROUP_CHUNKS = 8            # ~512KB per input load DMA
STORE_GROUP_ITEMS = 4            # ~1MB per output store DMA

MM_DT_NAME = os.environ.get("BASS_MM_DT", "float32r")
IMPL = os.environ.get("BASS_IMPL", "raw")

_program_cache: dict = {}


def _item_schedule(Q):
    """[(branch, chunk_col, item_idx)] in execution order; uniform across cores."""
    items = []
    col = 0
    for b in range(3):
        for _ in range(Q[b]):
            items.append((b, col, len(items)))
            col += R_BY_BRANCH[b]
    return items, col


def _build_program(Q: tuple):
    """One SPMD program for per-core tile counts Q=(Q0,Q1,Q2)."""
    items, c_total = _item_schedule(Q)
    n_items = len(items)

    nc = bacc.Bacc("TRN2", target_bir_lowering=False, debug=False)
    f32 = mybir.dt.float32
    mm_dt = getattr(mybir.dt, MM_DT_NAME)

    inp = nc.dram_tensor("inp", [P, c_total * P], mm_dt, kind="ExternalInput").ap()
    wch = nc.dram_tensor("wch", [P, N_WCHUNKS * E], mm_dt, kind="ExternalInput").ap()
    bias = nc.dram_tensor("bias", [P, 3 * E], f32, kind="ExternalInput").ap()
    out = nc.dram_tensor("out", [P, n_items * E], f32, kind="ExternalOutput").ap()

    # group items into input-load groups of <= LOAD_GROUP_CHUNKS chunks
    lgroups = []  # (item_lo, item_hi, col_lo, col_hi)
    lo = 0
    while lo < n_items:
        hi = lo
        while hi < n_items:
            r = R_BY_BRANCH[items[hi][0]]
            used = (items[hi][1] + r) - items[lo][1]
            if used > LOAD_GROUP_CHUNKS and hi > lo:
                break
            hi += 1
        col_lo = items[lo][1]
        col_hi = items[hi][1] if hi < n_items else c_total
        lgroups.append((lo, hi, col_lo, col_hi))
        lo = hi

    G = STORE_GROUP_ITEMS
    n_sgroups = math.ceil(n_items / G)

    with tile.TileContext(nc) as tc:
        with (
            tc.tile_pool(name="const", bufs=1) as cpool,
            tc.tile_pool(name="inp_g", bufs=len(lgroups)) as ipool,
            tc.tile_pool(name="psum", bufs=4, space="PSUM") as ppool,
            tc.tile_pool(name="outs", bufs=3) as opool,
        ):
            w_tile = cpool.tile([P, N_WCHUNKS * E], mm_dt, tag="w")
            b_tile = cpool.tile([P, 3 * E], f32, tag="b")
            nc.sync.dma_start(w_tile[:], wch[:])
            nc.sync.dma_start(b_tile[:], bias[:])

            in_tiles = []
            for (ilo, ihi, clo, chi) in lgroups:
                it = ipool.tile([P, (chi - clo) * P], mm_dt, tag="in")
                nc.sync.dma_start(it[:], inp[:, clo * P:chi * P])
                in_tiles.append(it)

            for sg in range(n_sgroups):
                g_lo = sg * G
                g_hi = min(g_lo + G, n_items)
                st = opool.tile([P, (g_hi - g_lo) * E], f32, tag="st")
                for i in range(g_lo, g_hi):
                    b, col, _ = items[i]
                    r = R_BY_BRANCH[b]
                    wb = WCHUNK_BASE[b]
                    gi = next(k for k, (lo2, hi2, _, _) in enumerate(lgroups)
                              if lo2 <= i < hi2)
                    it = in_tiles[gi]
                    cbase = col - lgroups[gi][2]
                    ps = ppool.tile([P, E], f32, tag="ps")
                    for cc in range(r):
                        nc.tensor.matmul(
                            ps[:],
                            it[:, (cbase + cc) * P:(cbase + cc + 1) * P],
                            w_tile[:, (wb + cc) * E:(wb + cc + 1) * E],
                            start=(cc == 0),
                            stop=(cc == r - 1),
                        )
                    nc.vector.tensor_add(
                        st[:, (i - g_lo) * E:(i - g_lo + 1) * E],
                        ps[:],
                        b_tile[:, b * E:(b + 1) * E],
                    )
                nc.scalar.dma_start(out[:, g_lo * E:g_hi * E], st[:])

    nc.compile()
    return nc


def _lgroups_of(items, n_items, c_total):
    """Group items into input-load groups of <= LOAD_GROUP_CHUNKS chunks."""
    lgroups = []
    lo = 0
    while lo < n_items:
        hi = lo
        while hi < n_items:
            r = R_BY_BRANCH[items[hi][0]]
            used = (items[hi][1] + r) - items[lo][1]
            if used > LOAD_GROUP_CHUNKS and hi > lo:
                break
            hi += 1
        col_lo = items[lo][1]
        col_hi = items[hi][1] if hi < n_items else c_total
        lgroups.append((lo, hi, col_lo, col_hi))
        lo = hi
    return lgroups


def _build_program_raw(Q: tuple):
    """Hand-synchronized SPMD program (no TileContext): minimal preamble/epilogue.

    Engine roles:
      sync   — input ring: branch weight chunks interleaved with input groups
      scalar — output ring: bias load first, then grouped stores
      tensor — per item: r accumulating matmuls (PSUM bank rotation)
      vector — per item: bias add PSUM -> staging slice
    """
    items, c_total = _item_schedule(Q)
    n_items = len(items)
    lgroups = _lgroups_of(items, n_items, c_total)
    G = STORE_GROUP_ITEMS
    n_sgroups = math.ceil(n_items / G)
    NPS = 6      # psum banks in rotation
    NSTG = 3     # staging buffers

    nc = bass.Bass("TRN2", target_bir_lowering=False, debug=False)
    f32 = mybir.dt.float32
    mm_dt = getattr(mybir.dt, MM_DT_NAME)

    inp = nc.dram_tensor("inp", [P, c_total * P], mm_dt, kind="ExternalInput").ap()
    wch = nc.dram_tensor("wch", [P, N_WCHUNKS * E], mm_dt, kind="ExternalInput").ap()
    bias = nc.dram_tensor("bias", [P, 3 * E], f32, kind="ExternalInput").ap()
    out = nc.dram_tensor("out", [P, n_items * E], f32, kind="ExternalOutput").ap()

    in_sb = nc.alloc_sbuf_tensor("in_sb", [P, c_total * P], mm_dt).ap()
    w_sb = nc.alloc_sbuf_tensor("w_sb", [P, N_WCHUNKS * E], mm_dt).ap()
    b_sb = nc.alloc_sbuf_tensor("b_sb", [P, 3 * E], f32).ap()
    stage = [
        nc.alloc_sbuf_tensor(f"stage{k}", [P, G * E], f32).ap() for k in range(NSTG)
    ]
    psum = [nc.alloc_psum_tensor(f"ps{k}", [P, E], f32).ap() for k in range(NPS)]

    # input ring schedule: weight chunk for a branch goes right before the
    # first group that contains an item of that branch
    ring_ops = []
    pos_of_group = {}
    pos_of_w = {}
    emitted_w = set()
    for gi, (ilo, ihi, clo, chi) in enumerate(lgroups):
        for i in range(ilo, ihi):
            b = items[i][0]
            if b not in emitted_w:
                pos_of_w[b] = len(ring_ops)
                ring_ops.append(("w", b))
                emitted_w.add(b)
        pos_of_group[gi] = len(ring_ops)
        ring_ops.append(("g", gi))

    group_of_item = {}
    for gi, (ilo, ihi, _, _) in enumerate(lgroups):
        for i in range(ilo, ihi):
            group_of_item[i] = gi

    # one semaphore per DMA op: concurrent DMAs inc per-SDMA-engine (16x +1),
    # so thresholds on a shared counter can fire before any single op is done
    with (
        nc.semaphore("sem_bias") as sem_bias,
        nc.semaphore("sem_mm") as sem_mm,
        nc.semaphore("sem_add") as sem_add,
        nc.Block() as block,
    ):
        sems_in = [nc.semaphore(f"sem_in{k}").__enter__() for k in range(len(ring_ops))]
        sems_st = [nc.semaphore(f"sem_st{k}").__enter__() for k in range(n_sgroups)]

        @block.sync
        def _(sync):
            for k, (kind, x) in enumerate(ring_ops):
                if kind == "w":
                    r = R_BY_BRANCH[x]
                    c0 = WCHUNK_BASE[x] * E
                    c1 = c0 + r * E
                    sync.dma_start(w_sb[:, c0:c1], wch[:, c0:c1]).then_inc(
                        sems_in[k], 16
                    )
                else:
                    _, _, clo, chi = lgroups[x]
                    sync.dma_start(
                        in_sb[:, clo * P:chi * P], inp[:, clo * P:chi * P]
                    ).then_inc(sems_in[k], 16)

        @block.tensor
        def _(tensor):
            waited = set()

            def need(pos):
                if pos not in waited:
                    tensor.wait_ge(sems_in[pos], 16)
                    waited.add(pos)

            for i, (b, col, _) in enumerate(items):
                r = R_BY_BRANCH[b]
                wb = WCHUNK_BASE[b]
                need(pos_of_w[b])
                need(pos_of_group[group_of_item[i]])
                if i >= NPS:
                    tensor.wait_ge(sem_add, i - NPS + 1)
                ps = psum[i % NPS]
                for cc in range(r):
                    mm = tensor.matmul(
                        ps[:],
                        in_sb[:, (col + cc) * P:(col + cc + 1) * P],
                        w_sb[:, (wb + cc) * E:(wb + cc + 1) * E],
                        start=(cc == 0),
                        stop=(cc == r - 1),
                    )
                mm.then_inc(sem_mm, 1)

        @block.vector
        def _(vector):
            vector.wait_ge(sem_bias, 16)
            for i, (b, col, _) in enumerate(items):
                sg, slot = divmod(i, G)
                vector.wait_ge(sem_mm, i + 1)
                if sg >= NSTG and slot == 0:
                    vector.wait_ge(sems_st[sg - NSTG], 16)
                vector.tensor_add(
                    stage[sg % NSTG][:, slot * E:(slot + 1) * E],
                    psum[i % NPS][:],
                    b_sb[:, b * E:(b + 1) * E],
                ).then_inc(sem_add, 1)

        @block.scalar
        def _(scalar):
            scalar.dma_start(b_sb[:], bias[:]).then_inc(sem_bias, 16)
            for sg in range(n_sgroups):
                g_lo = sg * G
                g_hi = min(g_lo + G, n_items)
                scalar.wait_ge(sem_add, g_hi)
                scalar.dma_start(
                    out[:, g_lo * E:g_hi * E],
                    stage[sg % NSTG][:, :(g_hi - g_lo) * E],
                ).then_inc(sems_st[sg], 16)
            for sg in range(n_sgroups):
                scalar.wait_ge(sems_st[sg], 16)

    return nc


def _get_program(Q: tuple):
    key = (tuple(Q), MM_DT_NAME, IMPL)
    if key not in _program_cache:
        if IMPL == "raw":
            _program_cache[key] = _build_program_raw(tuple(Q))
        else:
            _program_cache[key] = _build_program(tuple(Q))
    return _program_cache[key]


def _pack_weights(Ws, bs):
    wch = np.empty((P, N_WCHUNKS * E), dtype=np.float32)
    i = 0
    for b in range(3):
        Wb = np.asarray(Ws[b], dtype=np.float32)
        for cc in range(R_BY_BRANCH[b]):
            wch[:, i * E:(i + 1) * E] = Wb[:, cc * P:(cc + 1) * P].T
            i += 1
    bias = np.empty((P, 3 * E), dtype=np.float32)
    for b in range(3):
        bias[:, b * E:(b + 1) * E] = np.asarray(bs[b], dtype=np.float32)[None, :]
    return wch, bias


def kernel(signal, signal_mask, branch_idx, W0, b0, W1, b1, W2, b2):
    signal = np.asarray(signal, dtype=np.float32)
    signal_mask = np.asarray(signal_mask)
    branch_idx = np.asarray(branch_idx, dtype=np.int32)

    lengths = signal_mask.sum(axis=1, dtype=np.int64)
    w_of = np.asarray(WINDOW_SIZES, dtype=np.int64)[branch_idx]
    S_of = T // w_of
    n_valid = np.minimum(lengths // w_of, np.minimum(S_of, S_MAX))

    tokens = np.zeros((B, S_MAX, E), dtype=np.float32)
    padding_mask = np.arange(S_MAX, dtype=np.int64)[None, :] < n_valid[:, None]

    # ---- host packing: per branch, gather valid rows, split across cores ----
    Q = []
    scatter = [[] for _ in range(N_CORES)]   # per core: (item_start, flat_token_idx array)
    item_base = 0
    branch_blocks = [[] for _ in range(N_CORES)]  # chunks per core per branch, in order
    for b in range(3):
        w = WINDOW_SIZES[b]
        r = R_BY_BRANCH[b]
        samples = np.where(branch_idx == b)[0]
        nv = n_valid[samples]
        V = int(nv.sum())
        if V == 0:
            Q.append(0)
            continue
        sm = np.repeat(samples, nv)
        starts = np.cumsum(nv) - nv
        ss = np.arange(V, dtype=np.int64) - np.repeat(starts, nv)
        rpc = math.ceil(V / N_CORES)          # rows per core (last core may have fewer)
        Qb = math.ceil(rpc / P)
        Q.append(Qb)
        rows_pad = N_CORES * Qb * P
        # gather all valid rows: A[j] = signal[sm[j], ss[j]*w : ss[j]*w + w]
        A = signal[sm[:, None], (ss * w)[:, None] + np.arange(w)[None, :]]
        A = np.concatenate([A, np.zeros((rows_pad - V, w), np.float32)], axis=0)
        # rows are dealt to cores in contiguous blocks of Qb*P
        A = A.reshape(N_CORES, Qb, P, r, P)
        # chunk[core, tile, cc][k, j] = A[core, tile, j, cc, k]
        chunks = np.ascontiguousarray(A.transpose(0, 1, 3, 4, 2))  # (cores, Qb, r, Pk, Pj)
        gflat = sm * S_MAX + ss
        for c in range(N_CORES):
            branch_blocks[c].append(chunks[c].reshape(Qb * r, P, P))
            lo = min(c * Qb * P, V)
            hi = min((c + 1) * Qb * P, V)
            if hi > lo:
                scatter[c].append((item_base, gflat[lo:hi]))
        item_base += Qb
    Q = tuple(Q)
    n_items = sum(Q)
    if n_items == 0:
        return tokens, padding_mask

    c_total = Q[0] + 2 * Q[1] + 4 * Q[2]
    per_core_in = []
    for c in range(N_CORES):
        blk = np.concatenate(branch_blocks[c], axis=0)  # (c_total, P, P)
        per_core_in.append(
            np.ascontiguousarray(blk.transpose(1, 0, 2)).reshape(P, c_total * P)
        )

    wch, bias = _pack_weights((W0, W1, W2), (b0, b1, b2))

    nc = _get_program(Q)
    in_maps = [
        {"inp": per_core_in[c], "wch": wch, "bias": bias} for c in range(N_CORES)
    ]
    res = run_bass_kernel_spmd(nc, in_maps, list(range(N_CORES)))

    tok_flat = tokens.reshape(B * S_MAX, E)
    for c in range(N_CORES):
        out_c = res.results[c]["out"]   # (P, n_items*E): item i rows at [:, i*E:(i+1)*E]
        for item_start, gidx in scatter[c]:
            n = len(gidx)
            nt = (n + P - 1) // P
            blk = out_c[:, item_start * E:(item_start + nt) * E]
            rows = np.ascontiguousarray(
                blk.reshape(P, nt, E).transpose(1, 0, 2)
            ).reshape(nt * P, E)
            tok_flat[gidx] = rows[:n]
    return tokens, padding_mask
